# revision 13
# baseline (speedup 1.0000x reference)
"""Trainium2 Bass kernel for nn_Eq1to3 (eset_ops_1_to_3 + einsum broadcast expansion).

Reference computation (N=16, D=64, S=32, M=48, BASIS=4):
    t[b,n,s,m] = sum_d coefs[d,s,b] * x[n,d,m]        # tiny einsum
    out[n,s,i,j,k] = t0[n,s,i] + t1[n,s,j] + t2[n,s,k]
                     + (i==j==k) * t3[n,s,i] + bias[s]
Full output (16, 32, 48, 48, 48) f32 = 226.5 MB.

The harness tolerance (rel err 2e-2 of max|expected| ~13.7) leaves large
precision headroom, so the DEVICE output stream is bf16 (host upcasts after
gather): 14.16 MB/core -> ~40 us HBM-write floor at 358 GB/s vs ~79 us for
f32. Weights and x are bf16 too (PE matmuls ~4x faster than fp32, fp32 PSUM
accumulation); measured end-to-end rel err ~4e-3 << 2e-2 on the
deterministic harness inputs.

Sharding: data-parallel over N across 8 cores (2 batches/core). Per core the
output is [3072 rows p=(n,s,i), 2304 cols (j,k)]; partition q holds rows
p = 24q + r, so ns(q) = q//2, i(q,r) = 24*(q%2) + r. Host-prepared indicator
weights fold the batch index into the contraction (K = NL*D = 128):

    B_ps[q, (r,j)] = t0[ns(q), i(q,r)] + bias[s(q)] + t1[ns(q), j]   (PE, PSUM)
    T2[q, k]       = t2[ns(q), k]    T3[q, r] = t3[ns(q), i(q,r)]    (PE)

Producers (balanced so both finish with the DMA stream):
  - DVE:    out[q,(u,j,k)] = B[q,(u,j)] + T2G[q,k]  (1x fp32-path TT, bf16 out)
  - GpSimd: 4 of the 12 two-row groups (B mirrored to SBUF by ACT first)

Superdiagonal adds (dv = stride-(M+1) view of a group tile, 48 els/row):
a DVE op whose operands are ALL in SBUF blocks behind any concurrently
running GpSimd op (shared SBUF port) - measured stalls up to 8 us - while a
DVE op with one PSUM operand interleaves freely. So the DVE groups' diag
masks DGM[q,slot(r),s] = m3[q,r,s]*t3[q,r] are built into a PSUM tile by
ACT (per-row activation Copy with per-partition scale=t3), and DVE adds them
from PSUM; GpSimd groups build their own local SBUF dg (m3*t3) per group.
m3[q,(r,rr)] = one_hot(24*(q%2)+r)[rr], host-prepared in bf16.

Output DMA ring = producer engine (DVE groups -> SP ring, GpSimd -> ACT
ring): each ring's FIFO order matches its producer's completion order so a
late tile can never head-of-line-block the other stream. Row 0 goes out as
two half-rows to start the HBM stream ~1.3 us earlier; rows 22/23 as singles
to shrink the tail drain. Inputs: [x2|t2|t1] then [t0l0|t0l1|bias] bf16 on
the SP ring (T2's weights arrive ~0.7 us before chunk0's), [t3l0|t3l1] on
the ACT ring, m3 via GpSimd SWDGE.
"""

import numpy as np

N, D, S, M, BASIS = 16, 64, 32, 48, 4
N_CORES = 8
NL = N // N_CORES              # batches per core (2)
NS = NL * S                    # (n,s) groups per core (64)
ROWS = NS * M                  # output rows per core (3072)
JK = M * M                     # free size per row (2304)
P = 128                        # partitions
HALF = M // 2                  # rows per partition (24)
K = NL * D                     # contraction size (128)
HJ = JK // 2                   # half-row free size (1152)
# B-matmul chunks in r-rows (384 f32 = one PSUM bank per chunk)
CHUNKS = [8, 8, 8]
CHUNK_R0 = [sum(CHUNKS[:c]) for c in range(len(CHUNKS))]
GPS_2R0 = (8, 12, 16)          # two-row groups produced on GpSimd
GPS_1R0 = (20,)                # single rows produced on GpSimd
PE_ROWS = (18, 19, 21, 22, 23)  # rows produced directly by the PE
# (j0, nj) pieces of a PE row: free dim nj*48 <= 512 fp32 = one PSUM bank
PE_PIECES = [(0, 10), (10, 10), (20, 10), (30, 10), (40, 8)]
# rows whose diagonal is added on DVE from DGM_ps; slot = index here
ROWS_DVE = [0, 1, 2, 3, 4, 5, 6, 7, 10, 11, 14, 15] + list(PE_ROWS)
SLOT = {r: i for i, r in enumerate(ROWS_DVE)}

_PROG = None


def _build_prog():
    import concourse.bacc as bacc
    import concourse.tile as tile
    import concourse.mybir as mybir

    f32 = mybir.dt.float32
    bf16 = mybir.dt.bfloat16
    COPY = mybir.ActivationFunctionType.Copy
    nc = bacc.Bacc("TRN2", target_bir_lowering=False, debug=False,
                   num_devices=N_CORES)

    # xwa: [x2 | t2 | t1], xwb: [t0l0 | t0l1 | bias(row0, K=1 lhsT)]
    xwa_d = nc.dram_tensor("xwa", [K, M + 2 * P], bf16,
                           kind="ExternalInput").ap()
    xwb_d = nc.dram_tensor("xwb", [K, 3 * P], bf16, kind="ExternalInput").ap()
    w3_d = nc.dram_tensor("w3", [K, 2 * P], bf16, kind="ExternalInput").ap()
    m3_d = nc.dram_tensor("m3", [P, HALF * M], bf16, kind="ExternalInput").ap()
    y_d = nc.dram_tensor("y", [ROWS, JK], bf16, kind="ExternalOutput").ap()

    with tile.TileContext(nc) as tc:
        with (
            tc.tile_pool(name="const", bufs=1) as cpool,
            tc.tile_pool(name="psum", bufs=1, space="PSUM") as ppool,
            tc.tile_pool(name="outp", bufs=8) as opool,
            tc.tile_pool(name="bsb", bufs=4) as bpool,
            tc.tile_pool(name="pep", bufs=2, space="PSUM") as pepool,
        ):
            # ---- inputs: critical loads first on the SP HWDGE ring ----
            xwa_sb = cpool.tile([K, M + 2 * P], bf16)
            nc.sync.dma_start(out=xwa_sb[:], in_=xwa_d[:])
            xwb_sb = cpool.tile([K, 3 * P], bf16)
            nc.scalar.dma_start(out=xwb_sb[:], in_=xwb_d[:])
            w3_sb = cpool.tile([K, 2 * P], bf16)
            nc.scalar.dma_start(out=w3_sb[:], in_=w3_d[:])
            m3_sb = cpool.tile([P, HALF * M], bf16)
            nc.gpsimd.dma_start(out=m3_sb[:], in_=m3_d[:])
            ones_sb = cpool.tile([1, 1], bf16)
            nc.vector.memset(ones_sb[:], 1.0)

            x2_sb = xwa_sb[:, 0:M]
            w2_l = lambda: xwa_sb[:, M + 0 * P:M + 1 * P]
            w1_l = lambda: xwa_sb[:, M + 1 * P:M + 2 * P]
            w0_l = lambda li: xwb_sb[:, li * P:(li + 1) * P]
            wb_l = lambda: xwb_sb[:1, 2 * P:3 * P]
            w3_l = lambda li: w3_sb[:, li * P:(li + 1) * P]

            # ---- T2 first (every group needs it), then B chunk0, T3 ----
            # T2_ps shares the two rotating pepool banks with the PE row
            # pieces (it is released right after the T2G copy)
            T2_ps = pepool.tile([P, M], f32, tag="pp")
            nc.tensor.matmul(T2_ps[:], w2_l(), x2_sb[:], start=True, stop=True)
            T2G = cpool.tile([P, M], f32)
            # DVE copy: keeps ACT off the first-group critical path
            nc.vector.tensor_copy(T2G[:], T2_ps[:])

            # B[q, (r, j)]: accumulating matmuls, one PSUM bank per chunk
            # (PE-write + DVE-read in one PSUM bank is a HW fault, so chunks
            # must not share banks while groups stream)
            B_chunks = [ppool.tile([P, ci * M], f32, name=f"B_ps{c}")
                        for c, ci in enumerate(CHUNKS)]

            def emit_b_chunk(c):
                ci = CHUNKS[c]
                i0 = CHUNK_R0[c]
                blk = B_chunks[c].rearrange("q (r j) -> q r j", j=M)
                # t1 part: rhs[(n'd), (r, j)] = x[n', d, j]
                rhs = x2_sb[:, None, :].broadcast_to((K, ci, M))
                nc.tensor.matmul(blk, w1_l(), rhs, start=True, stop=False)
                for li in range(2):
                    # t0 part: rhs[(n'd), (r, j)] = x[n', d, 24*li + i0 + r]
                    rhs = x2_sb[:, HALF * li + i0:HALF * li + i0 + ci]
                    rhs = rhs[:, :, None].broadcast_to((K, ci, M))
                    nc.tensor.matmul(blk, w0_l(li), rhs,
                                     start=False, stop=False)
                # bias part: K=1 matmul against all-ones rhs
                rhs = ones_sb[0:1, 0:1].broadcast_to((1, ci, M))
                nc.tensor.matmul(blk, wb_l(), rhs, start=False, stop=True)

            emit_b_chunk(0)

            T3_ps = ppool.tile([P, HALF], f32)
            for li in range(2):
                nc.tensor.matmul(T3_ps[:], w3_l(li),
                                 x2_sb[:, HALF * li:HALF * (li + 1)],
                                 start=(li == 0), stop=(li == 1))
            T3G = cpool.tile([P, HALF], f32)    # ACT scale / gps mult source
            nc.scalar.activation(T3G[:], T3_ps[:], COPY)

            # ---- DGM_ps[q, slot, s] = m3[q, r(slot), s] * t3[q, r(slot)]
            # built by ACT (Copy with per-partition scale), lives in PSUM so
            # the DVE diag adds have a PSUM operand (no GpSimd port clash)
            m3_v = m3_sb.rearrange("q (r rr) -> q r rr", rr=M)
            DGM_ps = ppool.tile([P, len(ROWS_DVE) * M], f32)
            DGM_v = DGM_ps.rearrange("q (t s) -> q t s", s=M)

            def build_dgm(r):
                nc.scalar.activation(DGM_v[:, SLOT[r]], m3_v[:, r], COPY,
                                     scale=T3G[:, r:r + 1])

            build_dgm(0)
            build_dgm(1)

            # ---- groups: y row p = 24*q + r ----
            y_v = y_d.rearrange("(q r) f -> q r f", q=P)

            def chunk_of(r0):
                c = max(i for i, s in enumerate(CHUNK_R0) if s <= r0)
                return c, r0 - CHUNK_R0[c]

            def b_slice(r0, rw):
                c, ro = chunk_of(r0)
                B3 = B_chunks[c].rearrange("q (r j) -> q r j", j=M)
                return B3[:, ro:ro + rw, :]

            # GpSimd cannot read PSUM: ACT pre-mirrors all four gps B
            # slices as soon as their chunks exist, so no mirror ever queues
            # behind a blocking gps-ring DMA issue on the ACT sequencer
            gps_mirror = {}

            def premirror(r0):
                B_sb = bpool.tile([P, 2 * M], f32, tag="bsb")
                nc.scalar.activation(
                    B_sb.rearrange("q (r j) -> q r j", j=M),
                    b_slice(r0, 2), COPY)
                gps_mirror[r0] = B_sb

            def emit_group(r0, rw, on_gps=False):
                out_t = opool.tile([P, rw * JK], bf16, tag="out")
                o4 = out_t.rearrange("q (u j k) -> q u j k", u=rw, j=M)
                eng = nc.gpsimd if on_gps else nc.vector
                if on_gps:
                    in_j = gps_mirror[r0].rearrange("q (r j) -> q r j", j=M)
                else:
                    in_j = b_slice(r0, rw)
                in_j = in_j[:, :, :, None].broadcast_to((P, rw, M, M))
                in_k = T2G[:, None, None, :].broadcast_to((P, rw, M, M))
                eng.tensor_add(out=o4, in0=in_j, in1=in_k)
                # superdiagonal (free offsets u*2304 + s*49)
                dvv = out_t.rearrange("q (u f) -> q u f", u=rw)[:, :, ::M + 1]
                dvv = dvv[:, :, :M]
                if on_gps:
                    dg = bpool.tile([P, rw * M], bf16, tag="dg")
                    dg3 = dg.rearrange("q (u s) -> q u s", u=rw)
                    nc.gpsimd.tensor_mul(
                        out=dg3, in0=m3_v[:, r0:r0 + rw],
                        in1=T3G[:, r0:r0 + rw, None].broadcast_to((P, rw, M)))
                    nc.gpsimd.tensor_add(out=dvv, in0=dvv, in1=dg3)
                else:
                    t0s = SLOT[r0]
                    nc.vector.tensor_add(out=dvv, in0=dvv,
                                         in1=DGM_v[:, t0s:t0s + rw])
                dma_eng = nc.scalar if on_gps else nc.sync
                dma_eng.dma_start(
                    out=y_v[:, r0:r0 + rw, :],
                    in_=out_t.rearrange("q (u f) -> q u f", u=rw))

            def emit_half(h):
                # row 0 in two half-rows (j in [24h, 24h+24)) for an early
                # HBM stream start
                out_t = opool.tile([P, HJ], bf16, tag="out")
                o3 = out_t.rearrange("q (j k) -> q j k", k=M)
                in_j = b_slice(0, 1)[:, 0, HALF * h:HALF * (h + 1)]
                in_j = in_j[:, :, None].broadcast_to((P, HALF, M))
                in_k = T2G[:, None, :].broadcast_to((P, HALF, M))
                nc.vector.tensor_add(out=o3, in0=in_j, in1=in_k)
                # diagonal cells (j, k=24h+j): offsets 24h + j*49, j in [0,24)
                dvv = out_t[:, HALF * h:][:, ::M + 1][:, :HALF]
                nc.vector.tensor_add(
                    out=dvv, in0=dvv,
                    in1=DGM_v[:, SLOT[0], HALF * h:HALF * (h + 1)])
                nc.sync.dma_start(out=y_v[:, 0, HJ * h:HJ * (h + 1)],
                                  in_=out_t[:])

            emit_half(0)
            emit_half(1)
            emit_b_chunk(1)
            emit_b_chunk(2)
            for r0 in GPS_R0:
                premirror(r0)
            for r in ROWS_DVE[2:]:
                build_dgm(r)
            emit_group(1, 1)
            emit_group(8, 2, on_gps=True)
            emit_group(2, 2)
            emit_group(4, 2)
            emit_group(12, 2, on_gps=True)
            emit_group(6, 2)
            emit_group(16, 2, on_gps=True)
            emit_group(10, 2)
            emit_group(14, 2)
            emit_group(20, 2, on_gps=True)
            emit_group(18, 2)
            emit_group(22, 1)
            emit_group(23, 1)

    nc.compile()
    return nc


def _get_prog():
    global _PROG
    if _PROG is None:
        _PROG = _build_prog()
    return _PROG


def _make_in_maps(x, coefs, bias):
    import ml_dtypes

    bf = ml_dtypes.bfloat16
    x = np.asarray(x, dtype=np.float32)
    coefs = np.asarray(coefs, dtype=np.float32)
    bias = np.asarray(bias, dtype=np.float32)

    # partition q: ns(q) = q//2 = n*32 + s;  l(q) = q%2
    q = np.arange(P)
    n_of = q // 2 // S
    s_of = q // 2 % S
    # indicator weights w_b[(n',d), q] = coefs[d, s(q), b] * (n' == n(q))
    nd_n = np.repeat(np.arange(NL), D)                # (K,) n' of row
    nd_d = np.tile(np.arange(D), NL)                  # (K,) d of row
    sel = (nd_n[:, None] == n_of[None, :]).astype(np.float32)  # (K, P)

    def w_of(b):
        return coefs[nd_d[:, None], s_of[None, :], b] * sel

    lmask = [((q % 2) == li).astype(np.float32)[None, :] for li in range(2)]
    bias_row = np.zeros((K, P), np.float32)
    bias_row[0] = bias.reshape(S)[s_of]
    wb = [np.zeros((K, P), np.float32) for _ in range(2)]
    for li in range(2):
        wb[li] = w_of(0) * lmask[li]
    w3 = np.concatenate([w_of(3) * lmask[0], w_of(3) * lmask[1]], axis=1)
    w3 = np.ascontiguousarray(w3.astype(bf))

    # one-hot mask: m3[q, (r, rr)] = 1 iff rr == 24*(q%2) + r  (exact in bf16)
    i_of = HALF * (q % 2)[:, None] + np.arange(HALF)[None, :]
    m3 = np.zeros((P, HALF, M), np.float32)
    np.put_along_axis(m3, i_of[..., None], 1.0, axis=2)
    m3 = np.ascontiguousarray(m3.reshape(P, HALF * M).astype(bf))

    xwb = np.ascontiguousarray(
        np.concatenate([wb[0], wb[1], bias_row], axis=1).astype(bf))

    in_maps = []
    for core in range(N_CORES):
        x2 = x[NL * core:NL * (core + 1)].reshape(NL * D, M)
        xwa = np.ascontiguousarray(
            np.concatenate([x2, w_of(2), w_of(1)], axis=1).astype(bf))
        in_maps.append({"xwa": xwa, "xwb": xwb, "w3": w3, "m3": m3})
    return in_maps


def run(x, coefs, bias, **run_kwargs):
    """Run on hardware; returns (full_output, BassKernelResults)."""
    from concourse.bass_utils import run_bass_kernel_spmd

    prog = _get_prog()
    in_maps = _make_in_maps(x, coefs, bias)
    res = run_bass_kernel_spmd(prog, in_maps, list(range(N_CORES)), **run_kwargs)
    out = np.concatenate(
        [np.asarray(res.results[i]["y"]).astype(np.float32)
         .reshape(NL, S, M, M, M) for i in range(N_CORES)],
        axis=0)
    return out, res


def kernel(x, coefs, bias):
    out, _ = run(x, coefs, bias)
    return out


# revision 19
# speedup vs baseline: 1.0845x; 1.0845x over previous
"""Trainium2 Bass kernel for nn_Eq1to3 (eset_ops_1_to_3 + einsum broadcast expansion).

Reference computation (N=16, D=64, S=32, M=48, BASIS=4):
    t[b,n,s,m] = sum_d coefs[d,s,b] * x[n,d,m]        # tiny einsum
    out[n,s,i,j,k] = t0[n,s,i] + t1[n,s,j] + t2[n,s,k]
                     + (i==j==k) * t3[n,s,i] + bias[s]
Full output (16, 32, 48, 48, 48) f32 = 226.5 MB.

The harness tolerance (rel err 2e-2 of max|expected| ~13.7) leaves large
precision headroom, so the DEVICE output stream is bf16 (host upcasts after
gather): 14.16 MB/core -> ~40 us HBM-write floor at 358 GB/s vs ~79 us for
f32. Weights and x are bf16 too (PE matmuls ~4x faster than fp32, fp32 PSUM
accumulation); measured end-to-end rel err ~4e-3 << 2e-2 on the
deterministic harness inputs.

Sharding: data-parallel over N across 8 cores (2 batches/core). Per core the
output is [3072 rows p=(n,s,i), 2304 cols (j,k)]; partition q holds rows
p = 24q + r, so ns(q) = q//2, i(q,r) = 24*(q%2) + r. Host-prepared indicator
weights fold the batch index into the contraction (K = NL*D = 128):

    B_ps[q, (r,j)] = t0[ns(q), i(q,r)] + bias[s(q)] + t1[ns(q), j]   (PE, PSUM)
    T2[q, k]       = t2[ns(q), k]    T3[q, r] = t3[ns(q), i(q,r)]    (PE)

Producers (balanced so both finish with the DMA stream):
  - DVE:    out[q,(u,j,k)] = B[q,(u,j)] + T2G[q,k]  (1x fp32-path TT, bf16 out)
  - GpSimd: 4 of the 12 two-row groups (B mirrored to SBUF by ACT first)

Superdiagonal adds (dv = stride-(M+1) view of a group tile, 48 els/row):
a DVE op whose operands are ALL in SBUF blocks behind any concurrently
running GpSimd op (shared SBUF port) - measured stalls up to 8 us - while a
DVE op with one PSUM operand interleaves freely. So the DVE groups' diag
masks DGM[q,slot(r),s] = m3[q,r,s]*t3[q,r] are built into a PSUM tile by
ACT (per-row activation Copy with per-partition scale=t3), and DVE adds them
from PSUM; GpSimd groups build their own local SBUF dg (m3*t3) per group.
m3[q,(r,rr)] = one_hot(24*(q%2)+r)[rr], host-prepared in bf16.

Output DMA ring = producer engine (DVE groups -> SP ring, GpSimd -> ACT
ring): each ring's FIFO order matches its producer's completion order so a
late tile can never head-of-line-block the other stream. Row 0 goes out as
two half-rows to start the HBM stream ~1.3 us earlier; rows 22/23 as singles
to shrink the tail drain. Inputs: [x2|t2|t1] then [t0l0|t0l1|bias] bf16 on
the SP ring (T2's weights arrive ~0.7 us before chunk0's), [t3l0|t3l1] on
the ACT ring, m3 via GpSimd SWDGE.
"""

import numpy as np

N, D, S, M, BASIS = 16, 64, 32, 48, 4
N_CORES = 8
NL = N // N_CORES              # batches per core (2)
NS = NL * S                    # (n,s) groups per core (64)
ROWS = NS * M                  # output rows per core (3072)
JK = M * M                     # free size per row (2304)
P = 128                        # partitions
HALF = M // 2                  # rows per partition (24)
K = NL * D                     # contraction size (128)
HJ = JK // 2                   # half-row free size (1152)
# B-matmul chunks in r-rows (chunk free dim <= 512 f32 = one PSUM bank);
# rows 21-23 need no B (PE rows recompute their terms directly)
CHUNKS = [2, 8, 8, 3]
CHUNK_R0 = [sum(CHUNKS[:c]) for c in range(len(CHUNKS))]
GPS_2R0 = (8, 12, 16)          # two-row groups produced on GpSimd
GPS_1R0 = (20,)                # single rows produced on GpSimd
PE_ROWS = (18, 19, 21, 22, 23)  # rows produced directly by the PE
# (j0, nj) pieces of a PE row: free dim nj*48 <= 512 fp32 = one PSUM bank
PE_PIECES = [(0, 10), (10, 10), (20, 10), (30, 10), (40, 8)]
# rows whose diagonal is added on DVE from DGM_ps; slot = index here
ROWS_DVE = [0, 1, 2, 3, 4, 5, 6, 7, 10, 11, 14, 15] + list(PE_ROWS)
SLOT = {r: i for i, r in enumerate(ROWS_DVE)}

_PROG = None


def _build_prog():
    import concourse.bacc as bacc
    import concourse.tile as tile
    import concourse.mybir as mybir

    f32 = mybir.dt.float32
    bf16 = mybir.dt.bfloat16
    COPY = mybir.ActivationFunctionType.Copy
    nc = bacc.Bacc("TRN2", target_bir_lowering=False, debug=False,
                   num_devices=N_CORES)

    # xwa: [x2 | t2 | t1], xwb: [t0l0 | t0l1 | bias(row0, K=1 lhsT)]
    xwa_d = nc.dram_tensor("xwa", [K, M + 2 * P], bf16,
                           kind="ExternalInput").ap()
    xwb_d = nc.dram_tensor("xwb", [K, 3 * P], bf16, kind="ExternalInput").ap()
    w3_d = nc.dram_tensor("w3", [K, 2 * P], bf16, kind="ExternalInput").ap()
    m3_d = nc.dram_tensor("m3", [P, HALF * M], bf16, kind="ExternalInput").ap()
    y_d = nc.dram_tensor("y", [ROWS, JK], bf16, kind="ExternalOutput").ap()

    with tile.TileContext(nc) as tc:
        with (
            tc.tile_pool(name="const", bufs=1) as cpool,
            tc.tile_pool(name="psum", bufs=1, space="PSUM") as ppool,
            tc.tile_pool(name="outp", bufs=8) as opool,
            tc.tile_pool(name="bsb", bufs=4) as bpool,
            tc.tile_pool(name="pep", bufs=2, space="PSUM") as pepool,
        ):
            # ---- inputs: critical loads first on the SP HWDGE ring ----
            xwa_sb = cpool.tile([K, M + 2 * P], bf16)
            nc.sync.dma_start(out=xwa_sb[:], in_=xwa_d[:])
            xwb_sb = cpool.tile([K, 3 * P], bf16)
            nc.scalar.dma_start(out=xwb_sb[:], in_=xwb_d[:])
            w3_sb = cpool.tile([K, 2 * P], bf16)
            nc.scalar.dma_start(out=w3_sb[:], in_=w3_d[:])
            m3_sb = cpool.tile([P, HALF * M], bf16)
            nc.gpsimd.dma_start(out=m3_sb[:], in_=m3_d[:])
            ones_sb = cpool.tile([1, 1], bf16)
            nc.vector.memset(ones_sb[:], 1.0)
            # trigger ACT's lazy ACT_TABLE_LOAD (~1.3 us) during the input
            # loads instead of on the first real copy
            warm_sb = cpool.tile([1, 1], bf16)
            nc.scalar.activation(warm_sb[:], ones_sb[:], COPY)

            x2_sb = xwa_sb[:, 0:M]
            w2_l = lambda: xwa_sb[:, M + 0 * P:M + 1 * P]
            w1_l = lambda: xwa_sb[:, M + 1 * P:M + 2 * P]
            w0_l = lambda li: xwb_sb[:, li * P:(li + 1) * P]
            wb_l = lambda: xwb_sb[:1, 2 * P:3 * P]
            w3_l = lambda li: w3_sb[:, li * P:(li + 1) * P]

            # ---- T2 first (every group needs it), then B chunk0, T3 ----
            # T2_ps shares the two rotating pepool banks with the PE row
            # pieces (it is released right after the T2G copy)
            T2_ps = pepool.tile([P, M], f32, tag="pp")
            nc.tensor.matmul(T2_ps[:], w2_l(), x2_sb[:], start=True, stop=True)
            T2G = cpool.tile([P, M], f32)
            # DVE copy: keeps ACT off the first-group critical path
            nc.vector.tensor_copy(T2G[:], T2_ps[:])

            # B[q, (r, j)]: accumulating matmuls, one PSUM bank per chunk
            # (PE-write + DVE-read in one PSUM bank is a HW fault, so chunks
            # must not share banks while groups stream)
            B_chunks = [ppool.tile([P, ci * M], f32, name=f"B_ps{c}")
                        for c, ci in enumerate(CHUNKS)]

            def emit_b_chunk(c):
                ci = CHUNKS[c]
                i0 = CHUNK_R0[c]
                blk = B_chunks[c].rearrange("q (r j) -> q r j", j=M)
                # t1 part: rhs[(n'd), (r, j)] = x[n', d, j]
                rhs = x2_sb[:, None, :].broadcast_to((K, ci, M))
                nc.tensor.matmul(blk, w1_l(), rhs, start=True, stop=False)
                for li in range(2):
                    # t0 part: rhs[(n'd), (r, j)] = x[n', d, 24*li + i0 + r]
                    rhs = x2_sb[:, HALF * li + i0:HALF * li + i0 + ci]
                    rhs = rhs[:, :, None].broadcast_to((K, ci, M))
                    nc.tensor.matmul(blk, w0_l(li), rhs,
                                     start=False, stop=False)
                # bias part: K=1 matmul against all-ones rhs
                rhs = ones_sb[0:1, 0:1].broadcast_to((1, ci, M))
                nc.tensor.matmul(blk, wb_l(), rhs, start=False, stop=True)

            emit_b_chunk(0)

            # T3_ps takes the second pepool bank (released after T3G)
            T3_ps = pepool.tile([P, HALF], f32, tag="pp")
            for li in range(2):
                nc.tensor.matmul(T3_ps[:], w3_l(li),
                                 x2_sb[:, HALF * li:HALF * (li + 1)],
                                 start=(li == 0), stop=(li == 1))
            T3G = cpool.tile([P, HALF], f32)    # ACT scale / gps mult source
            nc.scalar.activation(T3G[:], T3_ps[:], COPY)

            # ---- DGM_ps[q, slot, s] = m3[q, r(slot), s] * t3[q, r(slot)]
            # built by ACT (Copy with per-partition scale), lives in PSUM so
            # the DVE diag adds have a PSUM operand (no GpSimd port clash)
            m3_v = m3_sb.rearrange("q (r rr) -> q r rr", rr=M)
            DGM_ps = ppool.tile([P, len(ROWS_DVE) * M], f32)
            DGM_v = DGM_ps.rearrange("q (t s) -> q t s", s=M)

            def build_dgm(r):
                nc.scalar.activation(DGM_v[:, SLOT[r]], m3_v[:, r], COPY,
                                     scale=T3G[:, r:r + 1])

            build_dgm(0)
            build_dgm(1)

            # ---- groups: y row p = 24*q + r ----
            y_v = y_d.rearrange("(q r) f -> q r f", q=P)

            def chunk_of(r0):
                c = max(i for i, s in enumerate(CHUNK_R0) if s <= r0)
                return c, r0 - CHUNK_R0[c]

            def b_slice(r0, rw):
                c, ro = chunk_of(r0)
                B3 = B_chunks[c].rearrange("q (r j) -> q r j", j=M)
                return B3[:, ro:ro + rw, :]

            # GpSimd cannot read PSUM: ACT pre-mirrors all four gps B
            # slices as soon as their chunks exist, so no mirror ever queues
            # behind a blocking gps-ring DMA issue on the ACT sequencer
            gps_mirror = {}

            def premirror(r0, rw):
                B_sb = bpool.tile([P, rw * M], f32, tag="bsb")
                nc.scalar.activation(
                    B_sb.rearrange("q (r j) -> q r j", j=M),
                    b_slice(r0, rw), COPY)
                gps_mirror[r0] = B_sb

            def emit_group(r0, rw, on_gps=False):
                out_t = opool.tile([P, rw * JK], bf16, tag="out")
                o4 = out_t.rearrange("q (u j k) -> q u j k", u=rw, j=M)
                eng = nc.gpsimd if on_gps else nc.vector
                if on_gps:
                    in_j = gps_mirror[r0].rearrange("q (r j) -> q r j", j=M)
                else:
                    in_j = b_slice(r0, rw)
                in_j = in_j[:, :, :, None].broadcast_to((P, rw, M, M))
                in_k = T2G[:, None, None, :].broadcast_to((P, rw, M, M))
                eng.tensor_add(out=o4, in0=in_j, in1=in_k)
                # superdiagonal (free offsets u*2304 + s*49)
                dvv = out_t.rearrange("q (u f) -> q u f", u=rw)[:, :, ::M + 1]
                dvv = dvv[:, :, :M]
                if on_gps:
                    dg = bpool.tile([P, rw * M], bf16, tag="dg")
                    dg3 = dg.rearrange("q (u s) -> q u s", u=rw)
                    nc.gpsimd.tensor_mul(
                        out=dg3, in0=m3_v[:, r0:r0 + rw],
                        in1=T3G[:, r0:r0 + rw, None].broadcast_to((P, rw, M)))
                    nc.gpsimd.tensor_add(out=dvv, in0=dvv, in1=dg3)
                else:
                    t0s = SLOT[r0]
                    nc.vector.tensor_add(out=dvv, in0=dvv,
                                         in1=DGM_v[:, t0s:t0s + rw])
                # gps groups drain on the ACT ring, except the last one:
                # its ACT-issued DMA would block the PE row-23 piece copies
                # queued behind it, so GpSimd issues that one itself (SWDGE)
                if on_gps:
                    dma_eng = nc.gpsimd if r0 in GPS_1R0 else nc.scalar
                else:
                    dma_eng = nc.sync
                dma_eng.dma_start(
                    out=y_v[:, r0:r0 + rw, :],
                    in_=out_t.rearrange("q (u f) -> q u f", u=rw))

            def emit_pe_row(r):
                # the PE computes a full output row (t0+t1+t2+bias) in five
                # one-bank PSUM pieces; ACT assembles them into SBUF; DVE
                # adds the diagonal
                row_t = opool.tile([P, JK], bf16, tag="out")
                for (j0, nj) in PE_PIECES:
                    pp = pepool.tile([P, nj * M], f32, tag="pp")
                    pv = pp.rearrange("q (j k) -> q j k", k=M)
                    rhs = x2_sb[:, j0:j0 + nj]
                    rhs = rhs[:, :, None].broadcast_to((K, nj, M))
                    nc.tensor.matmul(pv, w1_l(), rhs, start=True, stop=False)
                    rhs = x2_sb[:, None, :].broadcast_to((K, nj, M))
                    nc.tensor.matmul(pv, w2_l(), rhs, start=False, stop=False)
                    for li in range(2):
                        rhs = x2_sb[:, HALF * li + r:HALF * li + r + 1]
                        rhs = rhs[:, :, None].broadcast_to((K, nj, M))
                        nc.tensor.matmul(pv, w0_l(li), rhs,
                                         start=False, stop=False)
                    rhs = ones_sb[0:1, 0:1].broadcast_to((1, nj, M))
                    nc.tensor.matmul(pv, wb_l(), rhs, start=False, stop=True)
                    nc.scalar.activation(row_t[:, j0 * M:(j0 + nj) * M],
                                         pp[:], COPY)
                dvv = row_t[:, ::M + 1][:, :M]
                nc.vector.tensor_add(out=dvv, in0=dvv,
                                     in1=DGM_v[:, SLOT[r]])
                nc.sync.dma_start(out=y_v[:, r, :], in_=row_t[:])

            def emit_half(h):
                # row 0 in two half-rows (j in [24h, 24h+24)) for an early
                # HBM stream start
                out_t = opool.tile([P, HJ], bf16, tag="out")
                o3 = out_t.rearrange("q (j k) -> q j k", k=M)
                in_j = b_slice(0, 1)[:, 0, HALF * h:HALF * (h + 1)]
                in_j = in_j[:, :, None].broadcast_to((P, HALF, M))
                in_k = T2G[:, None, :].broadcast_to((P, HALF, M))
                nc.vector.tensor_add(out=o3, in0=in_j, in1=in_k)
                # diagonal cells (j, k=24h+j): offsets 24h + j*49, j in [0,24)
                dvv = out_t[:, HALF * h:][:, ::M + 1][:, :HALF]
                nc.vector.tensor_add(
                    out=dvv, in0=dvv,
                    in1=DGM_v[:, SLOT[0], HALF * h:HALF * (h + 1)])
                nc.sync.dma_start(out=y_v[:, 0, HJ * h:HJ * (h + 1)],
                                  in_=out_t[:])

            emit_half(0)
            emit_half(1)
            emit_b_chunk(1)
            emit_b_chunk(2)
            emit_b_chunk(3)
            for r0 in GPS_2R0:
                premirror(r0, 2)
            for r0 in GPS_1R0:
                premirror(r0, 1)
            for r in ROWS_DVE[2:]:
                build_dgm(r)
            emit_group(1, 1)
            emit_group(2, 2)
            emit_pe_row(18)
            emit_group(8, 2, on_gps=True)
            emit_group(4, 2)
            emit_pe_row(19)
            emit_group(12, 2, on_gps=True)
            emit_group(6, 2)
            emit_group(10, 2)
            emit_pe_row(21)
            emit_group(16, 2, on_gps=True)
            emit_group(14, 2)
            emit_pe_row(22)
            emit_pe_row(23)
            emit_group(20, 1, on_gps=True)

    nc.compile()
    return nc


def _get_prog():
    global _PROG
    if _PROG is None:
        _PROG = _build_prog()
    return _PROG


def _make_in_maps(x, coefs, bias):
    import ml_dtypes

    bf = ml_dtypes.bfloat16
    x = np.asarray(x, dtype=np.float32)
    coefs = np.asarray(coefs, dtype=np.float32)
    bias = np.asarray(bias, dtype=np.float32)

    # partition q: ns(q) = q//2 = n*32 + s;  l(q) = q%2
    q = np.arange(P)
    n_of = q // 2 // S
    s_of = q // 2 % S
    # indicator weights w_b[(n',d), q] = coefs[d, s(q), b] * (n' == n(q))
    nd_n = np.repeat(np.arange(NL), D)                # (K,) n' of row
    nd_d = np.tile(np.arange(D), NL)                  # (K,) d of row
    sel = (nd_n[:, None] == n_of[None, :]).astype(np.float32)  # (K, P)

    def w_of(b):
        return coefs[nd_d[:, None], s_of[None, :], b] * sel

    lmask = [((q % 2) == li).astype(np.float32)[None, :] for li in range(2)]
    bias_row = np.zeros((K, P), np.float32)
    bias_row[0] = bias.reshape(S)[s_of]
    wb = [np.zeros((K, P), np.float32) for _ in range(2)]
    for li in range(2):
        wb[li] = w_of(0) * lmask[li]
    w3 = np.concatenate([w_of(3) * lmask[0], w_of(3) * lmask[1]], axis=1)
    w3 = np.ascontiguousarray(w3.astype(bf))

    # one-hot mask: m3[q, (r, rr)] = 1 iff rr == 24*(q%2) + r  (exact in bf16)
    i_of = HALF * (q % 2)[:, None] + np.arange(HALF)[None, :]
    m3 = np.zeros((P, HALF, M), np.float32)
    np.put_along_axis(m3, i_of[..., None], 1.0, axis=2)
    m3 = np.ascontiguousarray(m3.reshape(P, HALF * M).astype(bf))

    xwb = np.ascontiguousarray(
        np.concatenate([wb[0], wb[1], bias_row], axis=1).astype(bf))

    in_maps = []
    for core in range(N_CORES):
        x2 = x[NL * core:NL * (core + 1)].reshape(NL * D, M)
        xwa = np.ascontiguousarray(
            np.concatenate([x2, w_of(2), w_of(1)], axis=1).astype(bf))
        in_maps.append({"xwa": xwa, "xwb": xwb, "w3": w3, "m3": m3})
    return in_maps


def run(x, coefs, bias, **run_kwargs):
    """Run on hardware; returns (full_output, BassKernelResults)."""
    from concourse.bass_utils import run_bass_kernel_spmd

    prog = _get_prog()
    in_maps = _make_in_maps(x, coefs, bias)
    res = run_bass_kernel_spmd(prog, in_maps, list(range(N_CORES)), **run_kwargs)
    out = np.concatenate(
        [np.asarray(res.results[i]["y"]).astype(np.float32)
         .reshape(NL, S, M, M, M) for i in range(N_CORES)],
        axis=0)
    return out, res


def kernel(x, coefs, bias):
    out, _ = run(x, coefs, bias)
    return out
